# revision 25
# baseline (speedup 1.0000x reference)
"""GraphNorm-style segmented normalization on 8 Trainium2 NeuronCores.

Strategy (x:[500000,256] f32, batch sorted int, 4096 graphs, params [256]):

- Host: graphs sorted by size (descending), dealt round-robin to 8 cores;
  slot k on every core holds that core's rank-(8k+c) graph, padded to the
  canonical size S_k = size(rank 8k) (rounded to even). Slot structure is
  identical across cores -> one SPMD Bass program, per-core data.
- Host packs each core's nodes channel-major and HALF-INTERLEAVED:
  xt[p, 2*w + h] = x[node w, h*128 + p]. A single bn_stats over a slot's
  [128, 2*S] range then yields independent stats for the lo channel half
  (even elements) and hi half (odd elements).
- Consecutive slot pairs with Spad <= 128 are padded to a COMMON size so one
  grouped bn_stats [128, 2, 2*Spad] covers both (amortizes DVE fixed cost;
  bn_stats free-size limit is 512).
- Device (per core, no PE/PSUM): per chunk: DMA load [128, 2W] f32 ->
  grouped bn_stats (DVE) -> batched stats math using
  E[(x-a*mu)^2] = E[x^2] + (a^2-2a)*mu^2 -> rstd via reciprocal+sqrt ->
  per-(slot,half) affine apply out = A*x + B written to a SEPARATE bf16
  tile, split across DVE (tensor_scalar), ACT (activation Identity) and
  GPSIMD (tensor_scalar) by a greedy 3-engine balance -> store bf16.
  Output in bf16 halves store traffic (128 MB -> 96 MB per core); output
  rounding is relative to the output value, so rel-err stays ~2^-9.
- Host un-interleaves, converts to f32, scatters rows back.
"""
import sys

if "/opt/trn_rl_repo" not in sys.path:
    sys.path.insert(0, "/opt/trn_rl_repo")

import numpy as np

import concourse.bacc as bacc
import concourse.tile as tile
from concourse import mybir
from concourse.bass_utils import run_bass_kernel_spmd

F32 = mybir.dt.float32
BF16 = mybir.dt.bfloat16
EPS = 1e-9
N_CORES = 8
H = 256
MINI_TGT = 1024     # nodes per mini-chunk (DMA/pipeline granule)
SUPER_MINIS = 4     # minis per super-chunk (stats-math batch granule)
X_BUFS = 16         # X alive ~3 supers (applies lag fronts by 2)
Y_BUFS = 10         # bf16 out tiles alive apply -> deferred store
PAIR_MAX = 0        # grouped bn_stats rejected by BIR verifier (out != 6/p)
PIPE_SKEW = 2       # supers of lag between fronts and applies
# measured per-op cost models (ns) for the apply split, stream = slot size S
DVE_APPLY_NS = lambda S: (174 + S) / 0.96
ACT_APPLY_NS = lambda S: (460 + S) / 1.2
GPS_APPLY_NS = lambda S: 330 + S / 0.72

_program_cache = {}
_last_run = None


def _plan_slots(sizes, n_cores):
    G = len(sizes)
    Gp = ((G + n_cores - 1) // n_cores) * n_cores
    sizes_p = np.concatenate([sizes, np.zeros(Gp - len(sizes), sizes.dtype)])
    order = np.argsort(-sizes_p, kind="stable")
    ranked = order.reshape(-1, n_cores)
    rank_sz = sizes_p[order].reshape(-1, n_cores)
    S = rank_sz[:, 0]
    keep = S > 0
    ranked = ranked[keep]
    S = S[keep].astype(np.int64)
    S = ((S + 1) // 2) * 2
    # group consecutive slots (uniform padded size) for batched bn_stats
    groups = []
    k = 0
    M = len(S)
    while k < M:
        if k + 1 < M and S[k] <= PAIR_MAX:
            S[k + 1] = S[k]
            groups.append((k, 2))
            k += 2
        else:
            groups.append((k, 1))
            k += 1
    offs = np.concatenate([[0], np.cumsum(S)])
    return ranked, S, offs, groups


def _plan_chunks(S, groups, w_tgt):
    """Group-aligned chunks of ~w_tgt nodes (slot ranges)."""
    chunks = []
    k0 = 0
    acc = 0
    for (g0, cnt) in groups:
        acc += int(S[g0]) * cnt
        if acc >= w_tgt:
            chunks.append((k0, g0 + cnt))
            k0 = g0 + cnt
            acc = 0
    if k0 < len(S):
        chunks.append((k0, len(S)))
    return chunks


def _plan_supers(minis, super_minis):
    return [minis[i:i + super_minis] for i in range(0, len(minis), super_minis)]


def _groups_in(groups, k0, k1):
    return [(g0, cnt) for (g0, cnt) in groups if k0 <= g0 < k1]


def _build_program(S, offs, groups, supers, M, Np):
    nc = bacc.Bacc("TRN2", target_bir_lowering=False, debug=False,
                   num_devices=N_CORES)
    xt_d = nc.dram_tensor("xt", [128, 2 * Np], F32, kind="ExternalInput")
    c1_d = nc.dram_tensor("c1", [128, M, 2], F32, kind="ExternalInput")
    c3_d = nc.dram_tensor("c3", [128, M, 2], F32, kind="ExternalInput")
    w_d = nc.dram_tensor("wp", [128, 2], F32, kind="ExternalInput")
    b_d = nc.dram_tensor("bp", [128, 2], F32, kind="ExternalInput")
    caa_d = nc.dram_tensor("caap", [128, 2], F32, kind="ExternalInput")
    nwa_d = nc.dram_tensor("nwap", [128, 2], F32, kind="ExternalInput")
    yt_d = nc.dram_tensor("yt", [128, 2 * Np], BF16, kind="ExternalOutput")

    mult = mybir.AluOpType.mult
    add = mybir.AluOpType.add

    with tile.TileContext(nc) as tc:
        with (
            tc.tile_pool(name="const", bufs=1) as constp,
            tc.tile_pool(name="xp", bufs=X_BUFS) as xp,
            tc.tile_pool(name="yp", bufs=Y_BUFS) as yp,
            tc.tile_pool(name="stp", bufs=2) as stp,
            tc.tile_pool(name="abp", bufs=2) as abp,
            tc.tile_pool(name="abp3", bufs=3) as abp3,
        ):
            c1t = constp.tile([128, M, 2], F32)
            c3t = constp.tile([128, M, 2], F32)
            wt = constp.tile([128, 2], F32)
            bt = constp.tile([128, 2], F32)
            caat = constp.tile([128, 2], F32)
            nwat = constp.tile([128, 2], F32)
            # consts ride the (otherwise idle at t=0) scalar ring so the
            # first X loads on the sync ring start immediately
            nc.scalar.dma_start(c1t[:], c1_d[:, :, :])
            nc.scalar.dma_start(c3t[:], c3_d[:, :, :])
            nc.scalar.dma_start(wt[:], w_d[:, :])
            nc.scalar.dma_start(bt[:], b_d[:, :])
            nc.scalar.dma_start(caat[:], caa_d[:, :])
            nc.scalar.dma_start(nwat[:], nwa_d[:, :])

            v = nc.vector

            def emit_front(super_):
                """Loads, grouped bn_stats, sigma^2 and 1/sigma^2 (DVE)."""
                k0 = super_[0][0]
                k1 = super_[-1][1]
                Mc = k1 - k0

                st = stp.tile([128, Mc, 6], F32, tag="st")
                Xs = []
                for (mk0, mk1) in super_:
                    n0 = int(offs[mk0])
                    n1 = int(offs[mk1])
                    X = xp.tile([128, 2 * (n1 - n0)], F32, tag="X")
                    nc.sync.dma_start(X[:], xt_d[:, 2 * n0:2 * n1])
                    Xs.append(X)
                    for (g0, cnt) in _groups_in(groups, mk0, mk1):
                        a = int(offs[g0]) - n0
                        sp = int(S[g0])
                        if cnt == 1:
                            nc.vector.bn_stats(st[:, g0 - k0, :],
                                               X[:, 2 * a:2 * (a + sp)])
                        else:
                            xi = X[:, 2 * a:2 * (a + cnt * sp)].rearrange(
                                "p (c w) -> p c w", c=cnt)
                            nc.vector.bn_stats(
                                st[:, g0 - k0:g0 - k0 + cnt, :], xi)

                # interleaved per-(slot,half) fields, [128, 2*Mc] views:
                st_r = st[:].rearrange("p m (x y) -> p (m x) y", x=2, y=3)
                m_v = st_r[:, :, 1]          # means  (lo,hi interleaved)
                v_v = st_r[:, :, 2]          # cnt*var
                c1s = c1t[:, k0:k1, :].rearrange("p m h -> p (m h)")
                c3s = c3t[:, k0:k1, :].rearrange("p m h -> p (m h)")

                U = 2 * Mc
                mu = abp.tile([128, U], F32, tag="mu")
                q = abp.tile([128, U], F32, tag="q")
                ex2 = abp.tile([128, U], F32, tag="ex2")
                sg = abp.tile([128, U], F32, tag="sg")

                v.tensor_tensor(mu[:], m_v, c1s, mult)          # mu
                v.tensor_tensor(q[:], m_v, m_v, mult)           # mean^2
                v.tensor_tensor(q[:], q[:], c1s, mult)          # *S/n
                v.tensor_tensor(ex2[:], v_v, c3s, mult)         # cnt*var/n
                v.tensor_tensor(ex2[:], ex2[:], q[:], add)      # E[x^2]
                v.tensor_tensor(q[:], mu[:], mu[:], mult)       # mu^2
                for h in (0, 1):
                    qh = q[:].rearrange("p (m h) -> p m h", h=2)[:, :, h]
                    sgh = sg[:].rearrange("p (m h) -> p m h", h=2)[:, :, h]
                    v.tensor_scalar(sgh, qh, caat[:, h:h + 1], EPS, mult, add)
                v.tensor_tensor(sg[:], sg[:], ex2[:], add)      # sigma^2+EPS
                v.reciprocal(sg[:], sg[:])                      # 1/sigma^2
                return [super_, Xs, mu, sg, None, None, k0]

            def emit_post(ctx):
                """rstd via ACT sqrt, then A/B (DVE) for a front-emitted
                super. Emitted AFTER an older super's applies so the sqrt
                never sits at ACT's queue head while DVE runs stats."""
                super_, Xs, mu, sg, _, _, k0 = ctx
                k1 = super_[-1][1]
                U = 2 * (k1 - k0)
                At = abp3.tile([128, U], F32, tag="At")
                Bt = abp3.tile([128, U], F32, tag="Bt")
                nc.scalar.sqrt(sg[:], sg[:])                    # rstd (ACT)
                v.tensor_tensor(Bt[:], mu[:], sg[:], mult)      # mu*rstd
                for h in (0, 1):
                    sgh = sg[:].rearrange("p (m h) -> p m h", h=2)[:, :, h]
                    Ah = At[:].rearrange("p (m h) -> p m h", h=2)[:, :, h]
                    Bh = Bt[:].rearrange("p (m h) -> p m h", h=2)[:, :, h]
                    v.tensor_scalar(Ah, sgh, wt[:, h:h + 1], None, mult)
                    v.tensor_scalar(Bh, Bh, nwat[:, h:h + 1], bt[:, h:h + 1],
                                    mult, add)
                ctx[4] = At
                ctx[5] = Bt
                return ctx

            def emit_applies(ctx, tail=False):
                """Apply (f32 X -> bf16 Y) + store for a super whose A/B math
                was emitted earlier. Each WHOLE mini goes to one engine
                (a shared output tile between engines would serialize them
                via Tile deps); three-way DVE/ACT/GPSIMD greedy balance.
                In the drain tail no stats run concurrently, so DVE is free
                to take a full share of applies."""
                super_, Xs, _, _, At, Bt, k0 = ctx
                k1 = super_[-1][1]
                U = 2 * (k1 - k0)
                if tail:
                    dve_load = 0.0
                else:
                    dve_load = sum(
                        (140 + 2 * int(S[g0]) * cnt) / 0.96
                        for (g0, cnt) in _groups_in(groups, k0, k1))
                    dve_load += 13 * (82 + U) / 0.96 + (82 + 6 * U) / 0.96
                act_load = (460 + U) / 1.2
                gps_load = 0.0
                stores = []
                for mi, (mk0, mk1) in enumerate(super_):
                    n0 = int(offs[mk0])
                    n1 = int(offs[mk1])
                    X = Xs[mi]
                    Y = yp.tile([128, 2 * (n1 - n0)], BF16, tag="Y")
                    Xr = X[:].rearrange("p (w h) -> p w h", h=2)
                    Yr = Y[:].rearrange("p (w h) -> p w h", h=2)
                    cd = sum(2 * DVE_APPLY_NS(int(S[k]))
                             for k in range(mk0, mk1))
                    ca = sum(2 * ACT_APPLY_NS(int(S[k]))
                             for k in range(mk0, mk1))
                    cg = sum(2 * GPS_APPLY_NS(int(S[k]))
                             for k in range(mk0, mk1))
                    opts = [(dve_load + cd, "dve"), (act_load + ca, "act"),
                            (gps_load + cg, "gps")]
                    opts.sort()
                    eng = opts[0][1]
                    if eng == "dve":
                        dve_load += cd
                    elif eng == "act":
                        act_load += ca
                    else:
                        gps_load += cg
                    for k in range(mk0, mk1):
                        a = int(offs[k]) - n0
                        s = int(S[k])
                        for h in (0, 1):
                            j2 = 2 * (k - k0) + h
                            xs = Xr[:, a:a + s, h]
                            ys = Yr[:, a:a + s, h]
                            Ac = At[:, j2:j2 + 1]
                            Bc = Bt[:, j2:j2 + 1]
                            if eng == "dve":
                                v.tensor_scalar(ys, xs, Ac, Bc, mult, add)
                            elif eng == "gps":
                                nc.gpsimd.tensor_scalar(ys, xs, Ac, Bc,
                                                        mult, add)
                            else:
                                nc.scalar.activation(
                                    ys, xs,
                                    mybir.ActivationFunctionType.Identity,
                                    bias=Bc, scale=Ac)
                    stores.append((n0, n1, Y, eng))
                return stores

            def emit_stores(stores):
                """Stores are deferred one super behind their applies so a
                store's semaphore wait never head-blocks the FIFO HWDGE ring
                (sync ring also carries the loads). ACT-applied minis store
                via ACT's own ring; DVE/GPS minis via sync (only SP and ACT
                have HWDGE rings)."""
                for (n0, n1, Y, eng) in stores:
                    if eng == "act":
                        nc.scalar.dma_start(yt_d[:, 2 * n0:2 * n1], Y[:])
                    else:
                        nc.sync.dma_start(yt_d[:, 2 * n0:2 * n1], Y[:])

            pend = []
            store_pend = []
            for super_ in supers:
                ctx = emit_front(super_)
                if len(pend) >= PIPE_SKEW:
                    if store_pend:
                        emit_stores(store_pend.pop(0))
                    store_pend.append(emit_applies(pend.pop(0)))
                pend.append(emit_post(ctx))
            while pend:
                if store_pend:
                    emit_stores(store_pend.pop(0))
                store_pend.append(emit_applies(pend.pop(0), tail=True))
            while store_pend:
                emit_stores(store_pend.pop(0))
    nc.compile()
    return nc


def _build_program_cached(S, offs, groups, supers, M, Np):
    key = (tuple(int(s) for s in S), tuple(groups),
           tuple(tuple(s) for s in supers), M, Np)
    nc = _program_cache.get(key)
    if nc is None:
        nc = _build_program(S, offs, groups, supers, M, Np)
        _program_cache[key] = nc
    return nc


def kernel(x, batch, alpha, weight, bias, num_graphs):
    global _last_run
    x = np.asarray(x, dtype=np.float32)
    batch = np.asarray(batch).astype(np.int64)
    alpha = np.asarray(alpha, dtype=np.float32)
    weight = np.asarray(weight, dtype=np.float32)
    bias = np.asarray(bias, dtype=np.float32)
    G = int(num_graphs)
    N, Hx = x.shape
    assert Hx == H

    sizes = np.bincount(batch, minlength=G).astype(np.int64)
    node_order = np.argsort(batch, kind="stable")
    gstarts = np.concatenate([[0], np.cumsum(sizes)])

    ranked, S, offs, groups = _plan_slots(sizes, N_CORES)
    M = len(S)
    Np = int(offs[-1])
    minis = _plan_chunks(S, groups, MINI_TGT)
    supers = _plan_supers(minis, SUPER_MINIS)

    nc = _build_program_cached(S, offs, groups, supers, M, Np)

    caa = alpha * alpha - 2.0 * alpha
    nwa = -(weight * alpha)
    w_p = np.ascontiguousarray(weight.reshape(2, 128).T)
    b_p = np.ascontiguousarray(bias.reshape(2, 128).T)
    caa_p = np.ascontiguousarray(caa.reshape(2, 128).T)
    nwa_p = np.ascontiguousarray(nwa.reshape(2, 128).T)

    xa = np.concatenate([x, np.zeros((1, H), np.float32)], axis=0)

    in_maps = []
    idx_per_core = []
    for c in range(N_CORES):
        gids = ranked[:, c]
        n = sizes[gids]
        idx = np.full(Np, N, dtype=np.int64)
        for k in range(M):
            g = gids[k]
            nk = int(n[k])
            if nk:
                idx[int(offs[k]):int(offs[k]) + nk] = \
                    node_order[gstarts[g]:gstarts[g] + nk]
        xp = xa[idx]                                   # [Np, 256]
        # xt[p, 2w+h] = xp[w, h*128+p]
        xv = xp.reshape(Np, 2, 128)
        xt = np.ascontiguousarray(xv.transpose(2, 0, 1)).reshape(128, 2 * Np)
        nguard = np.maximum(n, 1).astype(np.float32)
        c1 = (S.astype(np.float32) / nguard)
        c3 = (1.0 / nguard)
        c1b = np.broadcast_to(c1[None, :, None], (128, M, 2)).astype(
            np.float32).copy()
        c3b = np.broadcast_to(c3[None, :, None], (128, M, 2)).astype(
            np.float32).copy()
        in_maps.append({
            "xt": xt, "c1": c1b, "c3": c3b,
            "wp": w_p, "bp": b_p, "caap": caa_p, "nwap": nwa_p,
        })
        idx_per_core.append(idx)
    del xa

    _last_run = (nc, in_maps)
    res = run_bass_kernel_spmd(nc, in_maps, core_ids=list(range(N_CORES)))

    out = np.empty((N, H), dtype=np.float32)
    for c in range(N_CORES):
        yt = np.asarray(res.results[c]["yt"]).astype(np.float32)  # [128,2Np]
        yv = yt.reshape(128, Np, 2)
        # out_packed[w, h*128+p] = yv[p, w, h]
        yp_ = np.ascontiguousarray(yv.transpose(1, 2, 0)).reshape(Np, H)
        idx = idx_per_core[c]
        mask = idx < N
        out[idx[mask]] = yp_[mask]
    return out


# revision 26
# speedup vs baseline: 1.1470x; 1.1470x over previous
"""GraphNorm-style segmented normalization on 8 Trainium2 NeuronCores.

Strategy (x:[500000,256] f32, batch sorted int, 4096 graphs, params [256]):

- Host: graphs sorted by size (descending), dealt round-robin to 8 cores;
  slot k on every core holds that core's rank-(8k+c) graph, padded to the
  canonical size S_k = size(rank 8k) (rounded to even). Slot structure is
  identical across cores -> one SPMD Bass program, per-core data.
- Host packs each core's nodes channel-major and HALF-INTERLEAVED:
  xt[p, 2*w + h] = x[node w, h*128 + p]. A single bn_stats over a slot's
  [128, 2*S] range then yields independent stats for the lo channel half
  (even elements) and hi half (odd elements).
- Consecutive slot pairs with Spad <= 128 are padded to a COMMON size so one
  grouped bn_stats [128, 2, 2*Spad] covers both (amortizes DVE fixed cost;
  bn_stats free-size limit is 512).
- Device (per core, no PE/PSUM): per chunk: DMA load [128, 2W] f32 ->
  grouped bn_stats (DVE) -> batched stats math using
  E[(x-a*mu)^2] = E[x^2] + (a^2-2a)*mu^2 -> rstd via reciprocal+sqrt ->
  per-(slot,half) affine apply out = A*x + B written to a SEPARATE bf16
  tile, split across DVE (tensor_scalar), ACT (activation Identity) and
  GPSIMD (tensor_scalar) by a greedy 3-engine balance -> store bf16.
  Output in bf16 halves store traffic (128 MB -> 96 MB per core); output
  rounding is relative to the output value, so rel-err stays ~2^-9.
- Host un-interleaves, converts to f32, scatters rows back.
"""
import sys

if "/opt/trn_rl_repo" not in sys.path:
    sys.path.insert(0, "/opt/trn_rl_repo")

import numpy as np

import concourse.bacc as bacc
import concourse.tile as tile
from concourse import mybir
from concourse.bass_utils import run_bass_kernel_spmd

F32 = mybir.dt.float32
BF16 = mybir.dt.bfloat16
EPS = 1e-9
N_CORES = 8
H = 256
MINI_TGT = 1024     # nodes per mini-chunk (DMA/pipeline granule)
SUPER_MINIS = 4     # minis per super-chunk (stats-math batch granule)
X_BUFS = 18         # X alive ~4 supers (applies lag fronts by 2)
Y_BUFS = 6          # bf16 out tiles in flight (apply -> store)
PAIR_MAX = 0        # grouped bn_stats rejected by BIR verifier (out != 6/p)
PIPE_SKEW = 2       # supers of lag between fronts and applies
# measured per-op cost models (ns) for the apply split, stream = slot size S
DVE_APPLY_NS = lambda S: (174 + S) / 0.96
ACT_APPLY_NS = lambda S: (460 + S) / 1.2
GPS_APPLY_NS = lambda S: 330 + S / 0.72

_program_cache = {}
_last_run = None


def _plan_slots(sizes, n_cores):
    G = len(sizes)
    Gp = ((G + n_cores - 1) // n_cores) * n_cores
    sizes_p = np.concatenate([sizes, np.zeros(Gp - len(sizes), sizes.dtype)])
    order = np.argsort(-sizes_p, kind="stable")
    ranked = order.reshape(-1, n_cores)
    rank_sz = sizes_p[order].reshape(-1, n_cores)
    S = rank_sz[:, 0]
    keep = S > 0
    ranked = ranked[keep]
    S = S[keep].astype(np.int64)
    S = ((S + 1) // 2) * 2
    # group consecutive slots (uniform padded size) for batched bn_stats
    groups = []
    k = 0
    M = len(S)
    while k < M:
        if k + 1 < M and S[k] <= PAIR_MAX:
            S[k + 1] = S[k]
            groups.append((k, 2))
            k += 2
        else:
            groups.append((k, 1))
            k += 1
    offs = np.concatenate([[0], np.cumsum(S)])
    return ranked, S, offs, groups


def _plan_chunks(S, groups, w_tgt):
    """Group-aligned chunks of ~w_tgt nodes (slot ranges)."""
    chunks = []
    k0 = 0
    acc = 0
    for (g0, cnt) in groups:
        acc += int(S[g0]) * cnt
        if acc >= w_tgt:
            chunks.append((k0, g0 + cnt))
            k0 = g0 + cnt
            acc = 0
    if k0 < len(S):
        chunks.append((k0, len(S)))
    return chunks


def _plan_supers(minis, super_minis):
    return [minis[i:i + super_minis] for i in range(0, len(minis), super_minis)]


def _groups_in(groups, k0, k1):
    return [(g0, cnt) for (g0, cnt) in groups if k0 <= g0 < k1]


def _build_program(S, offs, groups, supers, M, Np):
    nc = bacc.Bacc("TRN2", target_bir_lowering=False, debug=False,
                   num_devices=N_CORES)
    xt_d = nc.dram_tensor("xt", [128, 2 * Np], F32, kind="ExternalInput")
    c1_d = nc.dram_tensor("c1", [128, M, 2], F32, kind="ExternalInput")
    c3_d = nc.dram_tensor("c3", [128, M, 2], F32, kind="ExternalInput")
    w_d = nc.dram_tensor("wp", [128, 2], F32, kind="ExternalInput")
    b_d = nc.dram_tensor("bp", [128, 2], F32, kind="ExternalInput")
    caa_d = nc.dram_tensor("caap", [128, 2], F32, kind="ExternalInput")
    nwa_d = nc.dram_tensor("nwap", [128, 2], F32, kind="ExternalInput")
    yt_d = nc.dram_tensor("yt", [128, 2 * Np], BF16, kind="ExternalOutput")

    mult = mybir.AluOpType.mult
    add = mybir.AluOpType.add

    with tile.TileContext(nc) as tc:
        with (
            tc.tile_pool(name="const", bufs=1) as constp,
            tc.tile_pool(name="xp", bufs=X_BUFS) as xp,
            tc.tile_pool(name="yp", bufs=Y_BUFS) as yp,
            tc.tile_pool(name="stp", bufs=2) as stp,
            tc.tile_pool(name="abp", bufs=2) as abp,
            tc.tile_pool(name="abp3", bufs=3) as abp3,
        ):
            c1t = constp.tile([128, M, 2], F32)
            c3t = constp.tile([128, M, 2], F32)
            wt = constp.tile([128, 2], F32)
            bt = constp.tile([128, 2], F32)
            caat = constp.tile([128, 2], F32)
            nwat = constp.tile([128, 2], F32)
            # consts ride the (otherwise idle at t=0) scalar ring so the
            # first X loads on the sync ring start immediately
            nc.scalar.dma_start(c1t[:], c1_d[:, :, :])
            nc.scalar.dma_start(c3t[:], c3_d[:, :, :])
            nc.scalar.dma_start(wt[:], w_d[:, :])
            nc.scalar.dma_start(bt[:], b_d[:, :])
            nc.scalar.dma_start(caat[:], caa_d[:, :])
            nc.scalar.dma_start(nwat[:], nwa_d[:, :])

            v = nc.vector

            def emit_front(super_):
                """Loads, grouped bn_stats, sigma^2 and 1/sigma^2 (DVE)."""
                k0 = super_[0][0]
                k1 = super_[-1][1]
                Mc = k1 - k0

                st = stp.tile([128, Mc, 6], F32, tag="st")
                Xs = []
                for (mk0, mk1) in super_:
                    n0 = int(offs[mk0])
                    n1 = int(offs[mk1])
                    X = xp.tile([128, 2 * (n1 - n0)], F32, tag="X")
                    nc.sync.dma_start(X[:], xt_d[:, 2 * n0:2 * n1])
                    Xs.append(X)
                    for (g0, cnt) in _groups_in(groups, mk0, mk1):
                        a = int(offs[g0]) - n0
                        sp = int(S[g0])
                        if cnt == 1:
                            nc.vector.bn_stats(st[:, g0 - k0, :],
                                               X[:, 2 * a:2 * (a + sp)])
                        else:
                            xi = X[:, 2 * a:2 * (a + cnt * sp)].rearrange(
                                "p (c w) -> p c w", c=cnt)
                            nc.vector.bn_stats(
                                st[:, g0 - k0:g0 - k0 + cnt, :], xi)

                # interleaved per-(slot,half) fields, [128, 2*Mc] views:
                st_r = st[:].rearrange("p m (x y) -> p (m x) y", x=2, y=3)
                m_v = st_r[:, :, 1]          # means  (lo,hi interleaved)
                v_v = st_r[:, :, 2]          # cnt*var
                c1s = c1t[:, k0:k1, :].rearrange("p m h -> p (m h)")
                c3s = c3t[:, k0:k1, :].rearrange("p m h -> p (m h)")

                U = 2 * Mc
                mu = abp.tile([128, U], F32, tag="mu")
                q = abp.tile([128, U], F32, tag="q")
                ex2 = abp.tile([128, U], F32, tag="ex2")
                sg = abp.tile([128, U], F32, tag="sg")

                v.tensor_tensor(mu[:], m_v, c1s, mult)          # mu
                v.tensor_tensor(q[:], m_v, m_v, mult)           # mean^2
                v.tensor_tensor(q[:], q[:], c1s, mult)          # *S/n
                v.tensor_tensor(ex2[:], v_v, c3s, mult)         # cnt*var/n
                v.tensor_tensor(ex2[:], ex2[:], q[:], add)      # E[x^2]
                v.tensor_tensor(q[:], mu[:], mu[:], mult)       # mu^2
                for h in (0, 1):
                    qh = q[:].rearrange("p (m h) -> p m h", h=2)[:, :, h]
                    sgh = sg[:].rearrange("p (m h) -> p m h", h=2)[:, :, h]
                    v.tensor_scalar(sgh, qh, caat[:, h:h + 1], EPS, mult, add)
                v.tensor_tensor(sg[:], sg[:], ex2[:], add)      # sigma^2+EPS
                v.reciprocal(sg[:], sg[:])                      # 1/sigma^2
                return [super_, Xs, mu, sg, None, None, k0]

            def emit_post(ctx):
                """rstd via ACT sqrt, then A/B (DVE) for a front-emitted
                super. Emitted AFTER an older super's applies so the sqrt
                never sits at ACT's queue head while DVE runs stats."""
                super_, Xs, mu, sg, _, _, k0 = ctx
                k1 = super_[-1][1]
                U = 2 * (k1 - k0)
                At = abp3.tile([128, U], F32, tag="At")
                Bt = abp3.tile([128, U], F32, tag="Bt")
                nc.scalar.sqrt(sg[:], sg[:])                    # rstd (ACT)
                v.tensor_tensor(Bt[:], mu[:], sg[:], mult)      # mu*rstd
                for h in (0, 1):
                    sgh = sg[:].rearrange("p (m h) -> p m h", h=2)[:, :, h]
                    Ah = At[:].rearrange("p (m h) -> p m h", h=2)[:, :, h]
                    Bh = Bt[:].rearrange("p (m h) -> p m h", h=2)[:, :, h]
                    v.tensor_scalar(Ah, sgh, wt[:, h:h + 1], None, mult)
                    v.tensor_scalar(Bh, Bh, nwat[:, h:h + 1], bt[:, h:h + 1],
                                    mult, add)
                ctx[4] = At
                ctx[5] = Bt
                return ctx

            def emit_applies(ctx, tail=False):
                """Apply (f32 X -> bf16 Y) + store for a super whose A/B math
                was emitted earlier. Each WHOLE mini goes to one engine
                (a shared output tile between engines would serialize them
                via Tile deps); three-way DVE/ACT/GPSIMD greedy balance.
                In the drain tail no stats run concurrently, so DVE is free
                to take a full share of applies."""
                super_, Xs, _, _, At, Bt, k0 = ctx
                k1 = super_[-1][1]
                U = 2 * (k1 - k0)
                if tail:
                    dve_load = 0.0
                else:
                    dve_load = sum(
                        (140 + 2 * int(S[g0]) * cnt) / 0.96
                        for (g0, cnt) in _groups_in(groups, k0, k1))
                    dve_load += 13 * (82 + U) / 0.96 + (82 + 6 * U) / 0.96
                act_load = (460 + U) / 1.2
                gps_load = 0.0
                for mi, (mk0, mk1) in enumerate(super_):
                    n0 = int(offs[mk0])
                    n1 = int(offs[mk1])
                    X = Xs[mi]
                    Y = yp.tile([128, 2 * (n1 - n0)], BF16, tag="Y")
                    Xr = X[:].rearrange("p (w h) -> p w h", h=2)
                    Yr = Y[:].rearrange("p (w h) -> p w h", h=2)
                    cd = sum(2 * DVE_APPLY_NS(int(S[k]))
                             for k in range(mk0, mk1))
                    ca = sum(2 * ACT_APPLY_NS(int(S[k]))
                             for k in range(mk0, mk1))
                    cg = sum(2 * GPS_APPLY_NS(int(S[k]))
                             for k in range(mk0, mk1))
                    opts = [(dve_load + cd, "dve"), (act_load + ca, "act"),
                            (gps_load + cg, "gps")]
                    opts.sort()
                    eng = opts[0][1]
                    if eng == "dve":
                        dve_load += cd
                    elif eng == "act":
                        act_load += ca
                    else:
                        gps_load += cg
                    for k in range(mk0, mk1):
                        a = int(offs[k]) - n0
                        s = int(S[k])
                        for h in (0, 1):
                            j2 = 2 * (k - k0) + h
                            xs = Xr[:, a:a + s, h]
                            ys = Yr[:, a:a + s, h]
                            Ac = At[:, j2:j2 + 1]
                            Bc = Bt[:, j2:j2 + 1]
                            if eng == "dve":
                                v.tensor_scalar(ys, xs, Ac, Bc, mult, add)
                            elif eng == "gps":
                                nc.gpsimd.tensor_scalar(ys, xs, Ac, Bc,
                                                        mult, add)
                            else:
                                nc.scalar.activation(
                                    ys, xs,
                                    mybir.ActivationFunctionType.Identity,
                                    bias=Bc, scale=Ac)
                    # ACT-applied minis store via ACT's own ring (self-dep,
                    # no sequencer wait); DVE/GPS minis ride sync (only SP
                    # and ACT have HWDGE rings).
                    if eng == "act":
                        nc.scalar.dma_start(yt_d[:, 2 * n0:2 * n1], Y[:])
                    else:
                        nc.sync.dma_start(yt_d[:, 2 * n0:2 * n1], Y[:])

            pend = []
            for super_ in supers:
                ctx = emit_front(super_)
                if len(pend) >= PIPE_SKEW:
                    emit_applies(pend.pop(0))
                pend.append(emit_post(ctx))
            while pend:
                emit_applies(pend.pop(0), tail=True)
    nc.compile()
    return nc


def _build_program_cached(S, offs, groups, supers, M, Np):
    key = (tuple(int(s) for s in S), tuple(groups),
           tuple(tuple(s) for s in supers), M, Np)
    nc = _program_cache.get(key)
    if nc is None:
        nc = _build_program(S, offs, groups, supers, M, Np)
        _program_cache[key] = nc
    return nc


def kernel(x, batch, alpha, weight, bias, num_graphs):
    global _last_run
    x = np.asarray(x, dtype=np.float32)
    batch = np.asarray(batch).astype(np.int64)
    alpha = np.asarray(alpha, dtype=np.float32)
    weight = np.asarray(weight, dtype=np.float32)
    bias = np.asarray(bias, dtype=np.float32)
    G = int(num_graphs)
    N, Hx = x.shape
    assert Hx == H

    sizes = np.bincount(batch, minlength=G).astype(np.int64)
    node_order = np.argsort(batch, kind="stable")
    gstarts = np.concatenate([[0], np.cumsum(sizes)])

    ranked, S, offs, groups = _plan_slots(sizes, N_CORES)
    M = len(S)
    Np = int(offs[-1])
    minis = _plan_chunks(S, groups, MINI_TGT)
    supers = _plan_supers(minis, SUPER_MINIS)

    nc = _build_program_cached(S, offs, groups, supers, M, Np)

    caa = alpha * alpha - 2.0 * alpha
    nwa = -(weight * alpha)
    w_p = np.ascontiguousarray(weight.reshape(2, 128).T)
    b_p = np.ascontiguousarray(bias.reshape(2, 128).T)
    caa_p = np.ascontiguousarray(caa.reshape(2, 128).T)
    nwa_p = np.ascontiguousarray(nwa.reshape(2, 128).T)

    xa = np.concatenate([x, np.zeros((1, H), np.float32)], axis=0)

    in_maps = []
    idx_per_core = []
    for c in range(N_CORES):
        gids = ranked[:, c]
        n = sizes[gids]
        idx = np.full(Np, N, dtype=np.int64)
        for k in range(M):
            g = gids[k]
            nk = int(n[k])
            if nk:
                idx[int(offs[k]):int(offs[k]) + nk] = \
                    node_order[gstarts[g]:gstarts[g] + nk]
        xp = xa[idx]                                   # [Np, 256]
        # xt[p, 2w+h] = xp[w, h*128+p]
        xv = xp.reshape(Np, 2, 128)
        xt = np.ascontiguousarray(xv.transpose(2, 0, 1)).reshape(128, 2 * Np)
        nguard = np.maximum(n, 1).astype(np.float32)
        c1 = (S.astype(np.float32) / nguard)
        c3 = (1.0 / nguard)
        c1b = np.broadcast_to(c1[None, :, None], (128, M, 2)).astype(
            np.float32).copy()
        c3b = np.broadcast_to(c3[None, :, None], (128, M, 2)).astype(
            np.float32).copy()
        in_maps.append({
            "xt": xt, "c1": c1b, "c3": c3b,
            "wp": w_p, "bp": b_p, "caap": caa_p, "nwap": nwa_p,
        })
        idx_per_core.append(idx)
    del xa

    _last_run = (nc, in_maps)
    res = run_bass_kernel_spmd(nc, in_maps, core_ids=list(range(N_CORES)))

    out = np.empty((N, H), dtype=np.float32)
    for c in range(N_CORES):
        yt = np.asarray(res.results[c]["yt"]).astype(np.float32)  # [128,2Np]
        yv = yt.reshape(128, Np, 2)
        # out_packed[w, h*128+p] = yv[p, w, h]
        yp_ = np.ascontiguousarray(yv.transpose(1, 2, 0)).reshape(Np, H)
        idx = idx_per_core[c]
        mask = idx < N
        out[idx[mask]] = yp_[mask]
    return out
